# revision 8
# baseline (speedup 1.0000x reference)
"""AttentionInteractionNetwork (GNN message passing) on 8 Trainium2 NeuronCores.

Sharding: edges assigned to cores by receiver node range (1250 nodes/core) and
receiver-sorted within each core, so the receiver segment-sum is fully local
and fused into the edge pipeline via one-hot matmuls accumulated in PSUM.
The sender segment-sum is computed from gate-scaled contributions written to
DRAM, re-gathered in sender-sorted order grouped by destination core,
aggregated the same way, exchanged with one AllToAll and reduced locally.
Node MLP is node-sharded.  All big matmuls run in float32r (TF32-like).
The first edge-MLP layer is factored through per-node tables
(nodes @ W0s.T / W0r.T, precomputed on-device) which are row-gathered per edge
and folded into the L0 PSUM with transpose-accumulate matmuls.
ssp(x) = softplus(x) - ln2 is computed as -Ln(Sigmoid(-x)); the sign flip and
the -ln2 shift are folded into the next layer's weights/biases on the host.
"""
import os
import time

import numpy as np

from contextlib import contextmanager

import concourse.bass as bass


@contextmanager
def _nullcm():
    yield None
import concourse.mybir as mybir
import concourse.tile as tile
from concourse.masks import make_identity

NC = 8
P = 128
D = 256
H = 512
N_NODES = 10000
N_EDGES = 200000
NLOC = N_NODES // NC          # 1250
WINS = (NLOC + P - 1) // P    # 10
NLOCP = WINS * P              # 1280
NPRE = ((N_NODES + 511) // 512) * 512  # 10240
LN2 = float(np.log(2.0))

f32 = mybir.dt.float32
f32r = mybir.dt.float32r
i32 = mybir.dt.int32
F = mybir.ActivationFunctionType
ALU = mybir.AluOpType
AXX = mybir.AxisListType.X


def _legalize_waits(nc, max_waits=1):
    """This walrus build accepts only ONE sync-wait per instruction; hoist the
    excess onto standalone event-semaphore waits on the same engine queue."""
    ctr = 0
    for f in nc.m.functions:
        for bb in f.blocks:
            out = []
            changed = False
            for ins in bb.instructions:
                si = ins.sync_info
                if si is not None and len(si.on_wait) > max_waits:
                    waits = list(si.on_wait)
                    extra, keep = waits[:-max_waits], waits[-max_waits:]
                    for w in extra:
                        ctr += 1
                        ev = mybir.InstEventSemaphore(
                            name=f"WLEG-{ctr}", ins=[], outs=[])
                        ev.engine = ins.engine
                        ev.sync_info = mybir.SyncInfo(on_wait=[w], on_update=[])
                        out.append(ev)
                    ins.sync_info = mybir.SyncInfo(
                        on_wait=keep, on_update=list(si.on_update))
                    changed = True
                out.append(ins)
            if changed:
                bb.instructions = out
    return ctr


# ---------------------------------------------------------------------------
def _host_prep(inputs):
    senders = np.asarray(inputs["senders"]).astype(np.int64)
    receivers = np.asarray(inputs["receivers"]).astype(np.int64)
    edges = np.asarray(inputs["edges"], dtype=np.float32)
    cutoff = np.asarray(inputs["cutoff"], dtype=np.float32).reshape(-1)
    nodes = np.asarray(inputs["nodes"], dtype=np.float32)

    owner = receivers // NLOC
    per_core = []
    win_lists = []
    for k in range(NC):
        idx = np.nonzero(owner == k)[0]
        idx = idx[np.argsort(receivers[idx], kind="stable")]
        per_core.append(idx)
        rloc = receivers[idx] - k * NLOC
        w = rloc // P
        win_lists.append([idx[w == wi] for wi in range(WINS)])

    B_w = [max(1, max((len(win_lists[k][wi]) + P - 1) // P for k in range(NC)))
           for wi in range(WINS)]
    B_w[-1] += (-sum(B_w)) % 4
    NB = sum(B_w)
    EPC = NB * P
    blk_win = np.concatenate(
        [np.full(B_w[w], w, np.int64) for w in range(WINS)])
    win_first = list(np.cumsum([0] + B_w[:-1]))
    win_last = list(np.cumsum(B_w) - 1)

    # pass-2 (sender) structure
    p2_lists = [[[None] * WINS for _ in range(NC)] for _ in range(NC)]
    B2 = np.zeros((NC, WINS), np.int64)
    for k in range(NC):
        idx = per_core[k]
        d = senders[idx] // NLOC
        sl = senders[idx] - d * NLOC
        w2 = sl // P
        for dd in range(NC):
            for wi in range(WINS):
                sel = idx[(d == dd) & (w2 == wi)]
                p2_lists[k][dd][wi] = sel
                B2[dd, wi] = max(B2[dd, wi], (len(sel) + P - 1) // P)
    B2 = np.maximum(B2, 1)
    B2[NC - 1, WINS - 1] += (-int(B2.sum())) % 4
    NB2 = int(B2.sum())
    EPC2 = NB2 * P
    blk2_dw = []
    dw_base_blk = {}
    for dd in range(NC):
        for wi in range(WINS):
            dw_base_blk[(dd, wi)] = len(blk2_dw)
            blk2_dw += [(dd, wi)] * int(B2[dd, wi])
    dw_first = {dw: None for dw in dw_base_blk}
    dw_last = {}
    for bi, dw in enumerate(blk2_dw):
        if dw_first[dw] is None:
            dw_first[dw] = bi
        dw_last[dw] = bi

    in_maps = []
    orig_pos_all = []
    for k in range(NC):
        idx = per_core[k]
        loc_order = np.full(EPC, -1, np.int64)
        for wi in range(WINS):
            base = win_first[wi] * P
            lst = win_lists[k][wi]
            loc_order[base:base + len(lst)] = lst
        valid = loc_order >= 0
        lo = loc_order[valid]
        pos_lookup = np.zeros(N_EDGES, np.int64)
        pos_lookup[lo] = np.nonzero(valid)[0]

        edges_rw = np.zeros((EPC, D), np.float32)
        edges_rw[valid] = edges[lo]
        cut = np.zeros(EPC, np.float32)
        cut[valid] = cutoff[lo]
        rrel = np.full(EPC, -1.0, np.float32)
        rrel[valid] = ((receivers[lo] - k * NLOC) % P).astype(np.float32)
        sidxn = np.zeros(EPC, np.int64)
        sidxn[valid] = senders[lo]
        ridxn = np.zeros(EPC, np.int64)
        ridxn[valid] = receivers[lo]

        g_idx = np.zeros(EPC2, np.int64)
        srel = np.full(EPC2, -1.0, np.float32)
        for dd in range(NC):
            for wi in range(WINS):
                base = dw_base_blk[(dd, wi)] * P
                eids = p2_lists[k][dd][wi]
                g_idx[base:base + len(eids)] = pos_lookup[eids]
                srel[base:base + len(eids)] = (
                    (senders[eids] - dd * NLOC) % P).astype(np.float32)

        nodes_loc = np.zeros((NLOCP, D), np.float32)
        nodes_loc[:NLOC] = nodes[k * NLOC:(k + 1) * NLOC]

        in_maps.append(dict(
            edges_rw=edges_rw,
            cut_col=np.ascontiguousarray(cut.reshape(NB, P).T),
            rrelT_col=np.ascontiguousarray(rrel.reshape(NB, P).T),
            sidxn_col=np.ascontiguousarray(sidxn.reshape(NB, P).T).astype(np.int32),
            ridxn_col=np.ascontiguousarray(ridxn.reshape(NB, P).T).astype(np.int32),
            gidx_col=np.ascontiguousarray(g_idx.reshape(NB2, P).T).astype(np.int32),
            srelT_col=np.ascontiguousarray(srel.reshape(NB2, P).T),
            nodes_loc=nodes_loc,
        ))
        orig_pos_all.append(loc_order)

    nodes_full = np.zeros((NPRE, D), np.float32)
    nodes_full[:N_NODES] = nodes

    def cols(b):
        return np.ascontiguousarray(b.reshape(-1, P).T)

    ew0 = np.asarray(inputs["ew0"], np.float32)
    ew1 = np.asarray(inputs["ew1"], np.float32)
    ew2 = np.asarray(inputs["ew2"], np.float32)
    nw0 = np.asarray(inputs["nw0"], np.float32)
    nw1 = np.asarray(inputs["nw1"], np.float32)
    nw2 = np.asarray(inputs["nw2"], np.float32)

    shared = dict(
        w0eT=np.ascontiguousarray(ew0[:, 0:D].T),
        wsrT=np.ascontiguousarray(
            np.concatenate([ew0[:, D:2 * D].T, ew0[:, 2 * D:3 * D].T], axis=1)),
        w1nT=np.ascontiguousarray((-ew1).T),
        w2nT=np.ascontiguousarray((-ew2).T),
        nw0T=np.ascontiguousarray(nw0.T),
        nw1nT=np.ascontiguousarray((-nw1).T),
        nw2nT=np.ascontiguousarray((-nw2).T),
        negb0e=cols(-np.asarray(inputs["eb0"], np.float32)),
        negb1e=cols(-(np.asarray(inputs["eb1"], np.float32) - LN2 * ew1.sum(1))),
        b2pe=cols(np.asarray(inputs["eb2"], np.float32) - LN2 * ew2.sum(1)),
        negb0n=cols(-np.asarray(inputs["nb0"], np.float32)),
        negb1n=cols(-(np.asarray(inputs["nb1"], np.float32) - LN2 * nw1.sum(1))),
        b2pn=cols(np.asarray(inputs["nb2"], np.float32) - LN2 * nw2.sum(1)),
        rwrow=np.asarray(inputs["rw"], np.float32).reshape(1, D),
        swrow=np.asarray(inputs["sw"], np.float32).reshape(1, D),
        egrow=np.asarray(inputs["eg"], np.float32).reshape(1, D),
        ebtrow=np.asarray(inputs["ebt"], np.float32).reshape(1, D),
        ngrow=np.asarray(inputs["ng"], np.float32).reshape(1, D),
        nbtrow=np.asarray(inputs["nbt"], np.float32).reshape(1, D),
        nodes_full=nodes_full,
    )
    for k in range(NC):
        in_maps[k].update(shared)

    meta = dict(
        EPC=EPC, NB=NB, EPC2=EPC2, NB2=NB2,
        blk_win=[int(x) for x in blk_win],
        win_first=[int(x) for x in win_first],
        win_last=[int(x) for x in win_last],
        blk2_dw=blk2_dw, dw_first=dw_first, dw_last=dw_last,
        rb=float(np.asarray(inputs["rb"]).reshape(-1)[0]),
        sb=float(np.asarray(inputs["sb"]).reshape(-1)[0]),
        triv_e=bool(np.all(np.asarray(inputs["eg"]) == 1.0)
                    and np.all(np.asarray(inputs["ebt"]) == 0.0)),
        triv_n=bool(np.all(np.asarray(inputs["ng"]) == 1.0)
                    and np.all(np.asarray(inputs["nbt"]) == 0.0)),
    )
    return meta, in_maps, orig_pos_all


# ---------------------------------------------------------------------------
def _emit_ln_block(nc, pool, src_ap, dst_fp32, g_rep, bt_rep, trivial):
    """LayerNorm over the free dim (256) of a [128, 256] block.
    src_ap may be PSUM or SBUF fp32; dst_fp32 is the normalized output."""
    m = pool.tile([P, 1], f32, tag="ln_m")
    nc.vector.reduce_sum(m, src_ap, axis=AXX)
    nc.vector.tensor_scalar_mul(m, m, 1.0 / D)
    xc = pool.tile([P, D], f32, tag="ln_xc")
    nc.vector.tensor_scalar_sub(xc, src_ap, m)
    sq = pool.tile([P, D], f32, tag="ln_sq")
    nc.vector.tensor_tensor(out=sq, in0=xc, in1=xc, op=ALU.mult)
    v = pool.tile([P, 1], f32, tag="ln_v")
    nc.vector.reduce_sum(v, sq, axis=AXX)
    nc.vector.tensor_scalar(v, v, 1.0 / D, 1e-5, op0=ALU.mult, op1=ALU.add)
    nc.scalar.sqrt(v, v)
    r = pool.tile([P, 1], f32, tag="ln_r")
    nc.vector.reciprocal(r, v)
    nc.vector.tensor_scalar_mul(dst_fp32, xc, r)
    if not trivial:
        nc.vector.tensor_tensor(out=dst_fp32, in0=dst_fp32, in1=g_rep, op=ALU.mult)
        nc.vector.tensor_tensor(out=dst_fp32, in0=dst_fp32, in1=bt_rep, op=ALU.add)


def _build_program(meta):
    EPC, NB = meta["EPC"], meta["NB"]
    EPC2, NB2 = meta["EPC2"], meta["NB2"]
    blk_win = meta["blk_win"]
    win_first, win_last = meta["win_first"], meta["win_last"]
    blk2_dw, dw_first, dw_last = meta["blk2_dw"], meta["dw_first"], meta["dw_last"]
    T, T2 = NB // 4, NB2 // 4
    rb_f, sb_f = meta["rb"], meta["sb"]
    triv_e, triv_n = meta["triv_e"], meta["triv_n"]

    nc = bass.Bass(num_devices=NC)

    edges_rw = nc.dram_tensor("edges_rw", [EPC, D], f32, kind="ExternalInput")
    cut_col = nc.dram_tensor("cut_col", [P, NB], f32, kind="ExternalInput")
    rrelT_col = nc.dram_tensor("rrelT_col", [P, NB], f32, kind="ExternalInput")
    sidxn_col = nc.dram_tensor("sidxn_col", [P, NB], i32, kind="ExternalInput")
    ridxn_col = nc.dram_tensor("ridxn_col", [P, NB], i32, kind="ExternalInput")
    gidx_col = nc.dram_tensor("gidx_col", [P, NB2], i32, kind="ExternalInput")
    srelT_col = nc.dram_tensor("srelT_col", [P, NB2], f32, kind="ExternalInput")
    nodes_loc = nc.dram_tensor("nodes_loc", [NLOCP, D], f32, kind="ExternalInput")
    nodes_full = nc.dram_tensor("nodes_full", [NPRE, D], f32, kind="ExternalInput")

    w0eT = nc.dram_tensor("w0eT", [D, H], f32, kind="ExternalInput")
    wsrT = nc.dram_tensor("wsrT", [D, 2 * H], f32, kind="ExternalInput")
    w1nT = nc.dram_tensor("w1nT", [H, H], f32, kind="ExternalInput")
    w2nT = nc.dram_tensor("w2nT", [H, D], f32, kind="ExternalInput")
    nw0T = nc.dram_tensor("nw0T", [3 * D, H], f32, kind="ExternalInput")
    nw1nT = nc.dram_tensor("nw1nT", [H, H], f32, kind="ExternalInput")
    nw2nT = nc.dram_tensor("nw2nT", [H, D], f32, kind="ExternalInput")
    negb0e = nc.dram_tensor("negb0e", [P, 4], f32, kind="ExternalInput")
    negb1e = nc.dram_tensor("negb1e", [P, 4], f32, kind="ExternalInput")
    b2pe = nc.dram_tensor("b2pe", [P, 2], f32, kind="ExternalInput")
    negb0n = nc.dram_tensor("negb0n", [P, 4], f32, kind="ExternalInput")
    negb1n = nc.dram_tensor("negb1n", [P, 4], f32, kind="ExternalInput")
    b2pn = nc.dram_tensor("b2pn", [P, 2], f32, kind="ExternalInput")
    rwrow = nc.dram_tensor("rwrow", [1, D], f32, kind="ExternalInput")
    swrow = nc.dram_tensor("swrow", [1, D], f32, kind="ExternalInput")
    egrow = nc.dram_tensor("egrow", [1, D], f32, kind="ExternalInput")
    ebtrow = nc.dram_tensor("ebtrow", [1, D], f32, kind="ExternalInput")
    ngrow = nc.dram_tensor("ngrow", [1, D], f32, kind="ExternalInput")
    nbtrow = nc.dram_tensor("nbtrow", [1, D], f32, kind="ExternalInput")

    nodeh_s = nc.dram_tensor("nodeh_s", [NPRE, H], f32r)
    nodeh_r = nc.dram_tensor("nodeh_r", [NPRE, H], f32r)
    contrib = nc.dram_tensor("contrib", [EPC, D], f32r)
    sentpart = nc.dram_tensor("sentpart", [NC * NLOCP, D], f32)
    a2aout = nc.dram_tensor("a2aout", [NC * NLOCP, D], f32)

    out_edges = nc.dram_tensor("out_edges", [EPC, D], f32, kind="ExternalOutput")
    out_nodes = nc.dram_tensor("out_nodes", [NLOCP, D], f32, kind="ExternalOutput")

    phases = os.environ.get("KPHASES", "ABCD")
    with tile.TileContext(nc) as tc:
        with tc.tile_pool(name="const", bufs=1) as const:
            ident = const.tile([P, P], f32)
            make_identity(nc, ident)
            identr = const.tile([P, P], f32r)
            nc.vector.tensor_copy(identr, ident)
            iota_ri = const.tile([P, P], i32)
            nc.gpsimd.iota(iota_ri, pattern=[[1, P]], channel_multiplier=0)
            iota_row = const.tile([P, P], f32)
            nc.vector.tensor_copy(iota_row, iota_ri)

            def load_w(name, dram, chunks, width):
                t = const.tile([P, chunks, width], f32r, tag=name)
                nc.sync.dma_start(
                    out=t, in_=dram.rearrange("(c p) m -> p c m", p=P).bitcast(f32r))
                return t

            w0e_sb = load_w("w0e", w0eT, 2, H)
            wsr_sb = load_w("wsr", wsrT, 2, 2 * H)
            w1_sb = load_w("w1", w1nT, 4, H)
            w2_sb = load_w("w2", w2nT, 4, D)
            nw0_sb = load_w("nw0", nw0T, 6, H)
            nw1_sb = load_w("nw1", nw1nT, 4, H)
            nw2_sb = load_w("nw2", nw2nT, 4, D)

            def load_sm(name, dram, w):
                t = const.tile([P, w], f32, tag=name)
                nc.sync.dma_start(out=t, in_=dram[:, :])
                return t

            nb0e_sb = load_sm("nb0e", negb0e, 4)
            nb1e_sb = load_sm("nb1e", negb1e, 4)
            b2e_sb = load_sm("b2e", b2pe, 2)
            nb0n_sb = load_sm("nb0n", negb0n, 4)
            nb1n_sb = load_sm("nb1n", negb1n, 4)
            b2n_sb = load_sm("b2n", b2pn, 2)

            def bcast(name, row):
                t = const.tile([P, D], f32, tag=name)
                nc.gpsimd.dma_start(out=t, in_=row[0:1, :].to_broadcast([P, D]))
                return t

            rw_rep = bcast("rw_rep", rwrow)
            sw_rep = bcast("sw_rep", swrow)
            eg_rep = bcast("eg_rep", egrow) if not triv_e else None
            ebt_rep = bcast("ebt_rep", ebtrow) if not triv_e else None
            ng_rep = bcast("ng_rep", ngrow) if not triv_n else None
            nbt_rep = bcast("nbt_rep", nbtrow) if not triv_n else None

            sidx_sb = const.tile([P, NB], i32)
            nc.sync.dma_start(out=sidx_sb, in_=sidxn_col[:, :])
            ridx_sb = const.tile([P, NB], i32)
            nc.sync.dma_start(out=ridx_sb, in_=ridxn_col[:, :])
            rrelT_sb = const.tile([P, NB], f32)
            nc.sync.dma_start(out=rrelT_sb, in_=rrelT_col[:, :])
            cut_sb = const.tile([P, NB], f32)
            nc.sync.dma_start(out=cut_sb, in_=cut_col[:, :])
            gidx_sb = const.tile([P, NB2], i32)
            nc.sync.dma_start(out=gidx_sb, in_=gidx_col[:, :])
            srelT_sb = const.tile([P, NB2], f32)
            nc.sync.dma_start(out=srelT_sb, in_=srelT_col[:, :])
            recv_agg = const.tile([P, WINS, D], f32)
            sent_agg = const.tile([P, WINS, D], f32)

            # ================= phase A: per-node h tables =================
            with tc.tile_pool(name="psA", bufs=6, space="PSUM") as psA, \
                 tc.tile_pool(name="sbA", bufs=3) as sbA:
                for t in range(NPRE // 512 if "A" in phases else 0):
                    nrow = sbA.tile([P, 4, D], f32, tag="nrow")
                    nc.sync.dma_start(
                        out=nrow,
                        in_=nodes_full[t * 512:(t + 1) * 512, :]
                            .rearrange("(b p) d -> p b d", p=P))
                    nT = sbA.tile([P, 2, 512], f32r, tag="nT")
                    for dc in range(2):
                        tp = psA.tile([P, 512], f32, tag="ps")
                        for b in range(4):
                            nc.tensor.transpose(
                                tp[:, b * P:(b + 1) * P],
                                nrow[:, b, dc * P:(dc + 1) * P], ident)
                        nc.vector.tensor_copy(nT[:, dc, :], tp)
                    for mg in range(2):
                        dst = nodeh_s if mg == 0 else nodeh_r
                        for nb in range(4):
                            op = psA.tile([P, 512], f32, tag="ps")
                            for kc in range(2):
                                nc.tensor.matmul(
                                    op, nT[:, kc, nb * P:(nb + 1) * P],
                                    wsr_sb[:, kc, mg * H:(mg + 1) * H],
                                    start=(kc == 0), stop=(kc == 1))
                            os_ = sbA.tile([P, H], f32r, tag="os")
                            nc.vector.tensor_copy(os_, op)
                            nc.sync.dma_start(
                                out=dst[t * 512 + nb * P:t * 512 + (nb + 1) * P, :],
                                in_=os_)

            # ================= phase B: edge pass =================
            with tc.tile_pool(name="psB", bufs=6, space="PSUM") as psB, \
                 tc.tile_pool(name="aggps", bufs=2, space="PSUM") as aggps, \
                 tc.tile_pool(name="sbB", bufs=2) as sbB, \
                 tc.tile_pool(name="lnp", bufs=4) as lnp:
                agg_tile = {}
                if "B" not in phases:
                    nc.vector.memset(recv_agg, 0.0)
                for t in range(T if "B" in phases else 0):
                    blks = [4 * t + i for i in range(4)]
                    xrow = sbB.tile([P, 4, D], f32, tag="xrow")
                    nc.sync.dma_start(
                        out=xrow,
                        in_=edges_rw[t * 512:(t + 1) * 512, :]
                            .rearrange("(b p) d -> p b d", p=P))

                    edT = sbB.tile([P, 2, 512], f32r, tag="edT")
                    for dc in range(2):
                        tp = psB.tile([P, 512], f32, tag="ps")
                        for b in range(4):
                            nc.tensor.transpose(
                                tp[:, b * P:(b + 1) * P],
                                xrow[:, b, dc * P:(dc + 1) * P], ident)
                        nc.vector.tensor_copy(edT[:, dc, :], tp)

                    grT = sbB.tile([P, 4], f32, tag="grT")
                    gsT = sbB.tile([P, 4], f32, tag="gsT")
                    scr = sbB.tile([P, D], f32, tag="scr")
                    for b in range(4):
                        nc.vector.tensor_tensor(out=scr, in0=xrow[:, b, :],
                                                in1=rw_rep, op=ALU.mult)
                        nc.vector.reduce_sum(grT[:, b:b + 1], scr, axis=AXX)
                        nc.vector.tensor_tensor(out=scr, in0=xrow[:, b, :],
                                                in1=sw_rep, op=ALU.mult)
                        nc.vector.reduce_sum(gsT[:, b:b + 1], scr, axis=AXX)
                    nc.scalar.activation(grT, grT, F.Sigmoid, bias=rb_f)
                    nc.scalar.activation(gsT, gsT, F.Sigmoid, bias=sb_f)
                    nc.vector.tensor_tensor(out=grT, in0=grT,
                                            in1=cut_sb[:, 4 * t:4 * t + 4], op=ALU.mult)
                    nc.vector.tensor_tensor(out=gsT, in0=gsT,
                                            in1=cut_sb[:, 4 * t:4 * t + 4], op=ALU.mult)

                    # ---- L0
                    l0 = [psB.tile([P, 512], f32, tag="ps", name=f"l0_{t}_{i}") for i in range(4)]
                    for mc in range(4):
                        for kc in range(2):
                            nc.tensor.matmul(
                                l0[mc], w0e_sb[:, kc, mc * P:(mc + 1) * P],
                                edT[:, kc, :], start=(kc == 0), stop=False)
                    for b in range(4):
                        gath = sbB.tile([P, H], f32r, tag="gath")
                        nc.gpsimd.indirect_dma_start(
                            out=gath, out_offset=None, in_=nodeh_s[:, :],
                            in_offset=bass.IndirectOffsetOnAxis(
                                ap=sidx_sb[:, blks[b]:blks[b] + 1], axis=0))
                        for hc in range(4):
                            nc.tensor.matmul(
                                l0[hc][:, b * P:(b + 1) * P].bitcast(f32r),
                                gath[:, hc * P:(hc + 1) * P],
                                identr,
                                is_transpose=True, start=False, stop=False)
                        gathr = sbB.tile([P, H], f32r, tag="gathr")
                        nc.gpsimd.indirect_dma_start(
                            out=gathr, out_offset=None, in_=nodeh_r[:, :],
                            in_offset=bass.IndirectOffsetOnAxis(
                                ap=ridx_sb[:, blks[b]:blks[b] + 1], axis=0))
                        for hc in range(4):
                            nc.tensor.matmul(
                                l0[hc][:, b * P:(b + 1) * P].bitcast(f32r),
                                gathr[:, hc * P:(hc + 1) * P],
                                identr,
                                is_transpose=True, start=False, stop=(b == 3))

                    g0 = sbB.tile([P, 4, 512], f32r, tag="g0")
                    sg = sbB.tile([P, 512], f32, tag="sg")
                    for hc in range(4):
                        nc.scalar.activation(sg, l0[hc], F.Sigmoid,
                                             bias=nb0e_sb[:, hc:hc + 1], scale=-1.0)
                        nc.scalar.activation(g0[:, hc, :], sg, F.Ln)

                    # ---- L1
                    l1 = [psB.tile([P, 512], f32, tag="ps", name=f"l1_{t}_{i}") for i in range(4)]
                    for mc in range(4):
                        for kc in range(4):
                            nc.tensor.matmul(
                                l1[mc], w1_sb[:, kc, mc * P:(mc + 1) * P],
                                g0[:, kc, :], start=(kc == 0), stop=(kc == 3))
                    g1 = sbB.tile([P, 4, 512], f32r, tag="g1")
                    sg1 = sbB.tile([P, 512], f32, tag="sg1")
                    for hc in range(4):
                        nc.scalar.activation(sg1, l1[hc], F.Sigmoid,
                                             bias=nb1e_sb[:, hc:hc + 1], scale=-1.0)
                        nc.scalar.activation(g1[:, hc, :], sg1, F.Ln)

                    # ---- L2 (feature-major out: [128d x 2, 512e])
                    z2 = sbB.tile([P, 2, 512], f32, tag="z2")
                    for mc in range(2):
                        l2 = psB.tile([P, 512], f32, tag="ps")
                        for kc in range(4):
                            nc.tensor.matmul(
                                l2, w2_sb[:, kc, mc * P:(mc + 1) * P],
                                g1[:, kc, :], start=(kc == 0), stop=(kc == 3))
                        nc.vector.tensor_scalar(z2[:, mc, :], l2,
                                                b2e_sb[:, mc:mc + 1], None,
                                                op0=ALU.add)

                    # ---- back to row-major, LN, residual, contribs, recv agg
                    outrow = sbB.tile([P, 4, D], f32, tag="outrow")
                    sentc = sbB.tile([P, 4, D], f32r, tag="sentc")
                    for j in range(2):  # pairs of e-blocks per psum bank
                        rp = psB.tile([P, 512], f32, tag="ps")
                        for half in range(2):
                            b = 2 * j + half
                            for dc in range(2):
                                nc.tensor.transpose(
                                    rp[:, half * D + dc * P:half * D + (dc + 1) * P],
                                    z2[:, dc, b * P:(b + 1) * P], ident)
                        for half in range(2):
                            b = 2 * j + half
                            upd = lnp.tile([P, D], f32, tag="upd")
                            _emit_ln_block(nc, lnp, rp[:, half * D:(half + 1) * D],
                                           upd, eg_rep, ebt_rep, triv_e)
                            nc.vector.tensor_tensor(out=outrow[:, b, :], in0=upd,
                                                    in1=xrow[:, b, :], op=ALU.add)
                            recvc = lnp.tile([P, D], f32r, tag="recvc")
                            nc.vector.tensor_scalar_mul(recvc, upd, grT[:, b:b + 1])
                            nc.vector.tensor_scalar_mul(sentc[:, b, :], upd,
                                                        gsT[:, b:b + 1])
                            # one-hot for this block
                            oh = lnp.tile([P, P], f32r, tag="oh")
                            nc.vector.tensor_tensor(
                                out=oh,
                                in0=rrelT_sb[:, blks[b]:blks[b] + 1].to_broadcast([P, P]),
                                in1=iota_row, op=ALU.is_equal)
                            w = blk_win[blks[b]]
                            if win_first[w] == blks[b]:
                                agg_tile[w] = aggps.tile([P, D], f32, tag="agg", name=f"agg_{w}")
                            nc.tensor.matmul(
                                agg_tile[w], oh, recvc,
                                start=(win_first[w] == blks[b]),
                                stop=(win_last[w] == blks[b]))
                            if win_last[w] == blks[b]:
                                nc.vector.tensor_copy(recv_agg[:, w, :], agg_tile[w])
                    nc.sync.dma_start(
                        out=out_edges[t * 512:(t + 1) * 512, :]
                            .rearrange("(b p) d -> p b d", p=P),
                        in_=outrow)
                    nc.sync.dma_start(
                        out=contrib[t * 512:(t + 1) * 512, :]
                            .rearrange("(b p) d -> p b d", p=P),
                        in_=sentc)

            # ================= phase C: sender aggregation =================
            with tc.tile_pool(name="psC", bufs=2, space="PSUM") as psC, \
                 tc.tile_pool(name="sbC", bufs=4) as sbC:
                agg2 = {}
                for b2 in range(NB2 if "C" in phases else 0):
                    dw = blk2_dw[b2]
                    gath2 = sbC.tile([P, D], f32r, tag="gath2")
                    nc.gpsimd.indirect_dma_start(
                        out=gath2, out_offset=None, in_=contrib[:, :],
                        in_offset=bass.IndirectOffsetOnAxis(
                            ap=gidx_sb[:, b2:b2 + 1], axis=0))
                    oh2 = sbC.tile([P, P], f32r, tag="oh2")
                    nc.vector.tensor_tensor(
                        out=oh2,
                        in0=srelT_sb[:, b2:b2 + 1].to_broadcast([P, P]),
                        in1=iota_row, op=ALU.is_equal)
                    if dw_first[dw] == b2:
                        agg2[dw] = psC.tile([P, D], f32, tag="agg2", name=f"agg2_{dw[0]}_{dw[1]}")
                    nc.tensor.matmul(agg2[dw], oh2, gath2,
                                     start=(dw_first[dw] == b2),
                                     stop=(dw_last[dw] == b2))
                    if dw_last[dw] == b2:
                        o = sbC.tile([P, D], f32, tag="aggout")
                        nc.vector.tensor_copy(o, agg2[dw])
                        d, w = dw
                        row0 = (d * WINS + w) * P
                        nc.sync.dma_start(out=sentpart[row0:row0 + P, :], in_=o)

                # AllToAll + local reduce
                if "C" not in phases:
                    nc.vector.memset(sent_agg, 0.0)
                if "C" in phases:
                    with tc.tile_critical():
                        with nc.semaphore() as sem:
                            nc.gpsimd.collective_compute(
                                "AllToAll", ALU.bypass,
                                replica_groups=[list(range(NC))],
                                ins=[sentpart[:]], outs=[a2aout[:]],
                            ).then_inc(sem, 1)
                            nc.gpsimd.wait_ge(sem, 1)
                a2av = a2aout.rearrange("(j w p) d -> w p j d", j=NC, p=P)
                for w in range(WINS if "C" in phases else 0):
                    acc = sbC.tile([P, NC, D], f32, tag="acc")
                    nc.sync.dma_start(out=acc, in_=a2av[w])
                    nc.vector.tensor_tensor(out=sent_agg[:, w, :], in0=acc[:, 0, :],
                                            in1=acc[:, 1, :], op=ALU.add)
                    for j in range(2, NC):
                        nc.vector.tensor_tensor(out=sent_agg[:, w, :],
                                                in0=sent_agg[:, w, :],
                                                in1=acc[:, j, :], op=ALU.add)

            # ================= phase D: node MLP =================
            with tc.tile_pool(name="psD", bufs=6, space="PSUM") as psD, \
                 tc.tile_pool(name="sbD", bufs=2) as sbD, \
                 tc.tile_pool(name="lnd", bufs=4) as lnd:
                sizes = []
                off = 0 if "D" in phases else NLOCP
                while off < NLOCP:
                    s = min(512, NLOCP - off)
                    sizes.append((off, s))
                    off += s
                for (off, S) in sizes:
                    nb = S // P
                    nrow = sbD.tile([P, 4, D], f32, tag="nrow")
                    nc.sync.dma_start(
                        out=nrow[:, :nb, :],
                        in_=nodes_loc[off:off + S, :]
                            .rearrange("(b p) d -> p b d", p=P))
                    featT = sbD.tile([P, 6, 512], f32r, tag="featT")
                    # nodes part (chunks 0-1)
                    for dc in range(2):
                        tp = psD.tile([P, 512], f32, tag="ps")
                        for b in range(nb):
                            nc.tensor.transpose(
                                tp[:, b * P:(b + 1) * P],
                                nrow[:, b, dc * P:(dc + 1) * P], ident)
                        nc.vector.tensor_copy(featT[:, dc, :S], tp[:, :S])
                    # recv_agg part (chunks 2-3), sent_agg part (chunks 4-5)
                    for (src, c0) in ((recv_agg, 2), (sent_agg, 4)):
                        for dc in range(2):
                            tp = psD.tile([P, 512], f32, tag="ps")
                            for b in range(nb):
                                w = off // P + b
                                nc.tensor.transpose(
                                    tp[:, b * P:(b + 1) * P],
                                    src[:, w, dc * P:(dc + 1) * P], ident)
                            nc.vector.tensor_copy(featT[:, c0 + dc, :S], tp[:, :S])

                    l0 = [psD.tile([P, 512], f32, tag="ps", name=f"nl0_{off}_{i}") for i in range(4)]
                    for mc in range(4):
                        for kc in range(6):
                            nc.tensor.matmul(
                                l0[mc][:, :S], nw0_sb[:, kc, mc * P:(mc + 1) * P],
                                featT[:, kc, :S], start=(kc == 0), stop=(kc == 5))
                    g0 = sbD.tile([P, 4, 512], f32r, tag="g0n")
                    sg = sbD.tile([P, 512], f32, tag="sgn")
                    for hc in range(4):
                        nc.scalar.activation(sg[:, :S], l0[hc][:, :S], F.Sigmoid,
                                             bias=nb0n_sb[:, hc:hc + 1], scale=-1.0)
                        nc.scalar.activation(g0[:, hc, :S], sg[:, :S], F.Ln)
                    l1 = [psD.tile([P, 512], f32, tag="ps", name=f"nl1_{off}_{i}") for i in range(4)]
                    for mc in range(4):
                        for kc in range(4):
                            nc.tensor.matmul(
                                l1[mc][:, :S], nw1_sb[:, kc, mc * P:(mc + 1) * P],
                                g0[:, kc, :S], start=(kc == 0), stop=(kc == 3))
                    g1 = sbD.tile([P, 4, 512], f32r, tag="g1n")
                    sg1 = sbD.tile([P, 512], f32, tag="sg1n")
                    for hc in range(4):
                        nc.scalar.activation(sg1[:, :S], l1[hc][:, :S], F.Sigmoid,
                                             bias=nb1n_sb[:, hc:hc + 1], scale=-1.0)
                        nc.scalar.activation(g1[:, hc, :S], sg1[:, :S], F.Ln)
                    z2 = sbD.tile([P, 2, 512], f32, tag="z2n")
                    for mc in range(2):
                        l2 = psD.tile([P, 512], f32, tag="ps")
                        for kc in range(4):
                            nc.tensor.matmul(
                                l2[:, :S], nw2_sb[:, kc, mc * P:(mc + 1) * P],
                                g1[:, kc, :S], start=(kc == 0), stop=(kc == 3))
                        nc.vector.tensor_scalar(z2[:, mc, :S], l2[:, :S],
                                                b2n_sb[:, mc:mc + 1], None,
                                                op0=ALU.add)
                    outn = sbD.tile([P, 4, D], f32, tag="outn")
                    for j in range((nb + 1) // 2):
                        rp = psD.tile([P, 512], f32, tag="ps")
                        nh = min(2, nb - 2 * j)
                        for half in range(nh):
                            b = 2 * j + half
                            for dc in range(2):
                                nc.tensor.transpose(
                                    rp[:, half * D + dc * P:half * D + (dc + 1) * P],
                                    z2[:, dc, b * P:(b + 1) * P], ident)
                        for half in range(nh):
                            b = 2 * j + half
                            upd = lnd.tile([P, D], f32, tag="updn")
                            _emit_ln_block(nc, lnd, rp[:, half * D:(half + 1) * D],
                                           upd, ng_rep, nbt_rep, triv_n)
                            nc.vector.tensor_tensor(out=outn[:, b, :], in0=upd,
                                                    in1=nrow[:, b, :], op=ALU.add)
                    nc.sync.dma_start(
                        out=out_nodes[off:off + S, :]
                            .rearrange("(b p) d -> p b d", p=P),
                        in_=outn[:, :nb, :])
    return nc


# ---------------------------------------------------------------------------
def _run(nc, in_maps):
    """Execute via PJRT (axon) with optional benchmark repeats."""
    from concourse import bass2jax
    reps = int(os.environ.get("KERNEL_BENCH_REPS", "0"))
    if reps <= 0:
        return bass2jax.run_bass_via_pjrt(nc, in_maps, n_cores=NC)
    res = bass2jax.run_bass_via_pjrt(nc, in_maps, n_cores=NC)
    times = []
    for _ in range(reps):
        t0 = time.monotonic()
        res = bass2jax.run_bass_via_pjrt(nc, in_maps, n_cores=NC)
        times.append(time.monotonic() - t0)
    print("KERNEL_BENCH wall times (s):", [f"{t:.4f}" for t in times])
    return res


def kernel(**inputs):
    meta, in_maps, orig_pos_all = _host_prep(inputs)
    nc = _build_program(meta)
    _legalize_waits(nc)
    results = _run(nc, in_maps)

    nodes = np.asarray(inputs["nodes"], dtype=np.float32)
    edges = np.asarray(inputs["edges"], dtype=np.float32)
    nodes_out = np.empty_like(nodes)
    edges_out = np.empty_like(edges)
    for k in range(NC):
        nodes_out[k * NLOC:(k + 1) * NLOC] = results[k]["out_nodes"][:NLOC]
        loc = orig_pos_all[k]
        valid = loc >= 0
        edges_out[loc[valid]] = results[k]["out_edges"][valid]
    return nodes_out, edges_out
